# revision 29
# baseline (speedup 1.0000x reference)
"""Trainium2 Bass kernel for nn_CLFormer (3-block linear-attention transformer).

Sharding: pure data parallel — batch 32 split as 4 per NeuronCore across 8
cores; all parameters replicated; outputs concatenated.

Per-core layout: 4 batches x 32 channels packed on the 128 SBUF partitions
("channel-major" [128=4bx32c, L]). The kv-gram contracts over tokens, so a
token-major copy is produced per block by SBUF->SBUF DMA transposes (block
0 from a bf16 cast of x loaded via casting DMA; blocks 1-2 from the gelu
output during the previous phase 2 — no DRAM round trip, no PE transposes).
q returns to channel-major the same way. The k-softmax denominator comes
from a 1-moving-row matmul (et stationary x ones) accumulated next to the
gram. Attention-out and FC1 fuse into M1 = diag(1/ksum) @ G @ W1; FC
matmuls run as single 128-wide matmuls with block-diagonal weights. Phase 1
of block b+1 is issued inside block b's phase 2 so all engines stay busy
across block boundaries. Replicated/derived parameters (weight strips, BN
eval folding) are precomputed on the host.
"""
import sys
import numpy as np

for _p in ("/opt/trn_rl_repo", "/root/.axon_site/_ro/trn_rl_repo"):
    if _p not in sys.path:
        sys.path.append(_p)

from contextlib import ExitStack

import concourse.bass as bass
import concourse.mybir as mybir
import bass_rust
from bass_rust import add_dep_helper
from concourse import tile
from concourse.masks import make_identity
from concourse.bass_utils import run_bass_kernel_spmd

F32 = mybir.dt.float32
BF16 = mybir.dt.bfloat16
AF = mybir.ActivationFunctionType
MUL = mybir.AluOpType.mult
ADD = mybir.AluOpType.add

P = 128
B_LOC = 4            # batches per core
C = 32               # channels
L = 16384            # sequence length
NB = 3               # transformer blocks
DOUT = 10
HEADS = 4
DH = 8
BN_EPS = 1e-5

SLC = 4096           # slice width (tokens per phase-1 slice)
NSL = L // SLC       # 4 slices
NCH = SLC // 128     # 32 chunks per slice
HCH = NCH // 2       # chunks per half slice
ZW = 1024            # phase-2 tile width (tokens per z/gelu tile)
NZ = L // ZW         # 16 phase-2 steps
DEBUG = False
DBG_BLK = 0


# ---------------------------------------------------------------- waitfix --
_WF_SKIP = {"InstEventSemaphore"}
_wf_ctr = [0]


def _fix_sync_waits(nc):
    """Hoist excess sync waits onto InstEventSemaphore (this walrus build
    accepts only 1 wait per instruction). The event-sem executes on the same
    engine stream immediately before, preserving semantics."""
    for fn in nc.m.functions:
        new_blocks = []
        for blk in fn.blocks:
            out = []
            for ins in blk.instructions:
                tname = type(ins).__name__
                si = ins.sync_info
                if si is None or tname in _WF_SKIP:
                    out.append(ins)
                    continue
                waits = list(si.on_wait)
                if len(waits) <= 1:
                    out.append(ins)
                    continue
                keep = waits[-1:]
                excess = waits[:-1]
                for i in range(0, len(excess), 2):
                    chunk = excess[i:i + 2]
                    _wf_ctr[0] += 1
                    ev = mybir.InstEventSemaphore(
                        name=f"wfix{_wf_ctr[0]}", ins=[], outs=[])
                    ev.engine = ins.engine
                    ev.sync_info = mybir.SyncInfo(on_wait=chunk, on_update=[])
                    out.append(ev)
                ins.sync_info = mybir.SyncInfo(
                    on_wait=keep, on_update=list(si.on_update))
                out.append(ins)
            nb = bass_rust.BasicBlock(name=blk.name, instructions=out)
            new_blocks.append(nb)
        fn.blocks = new_blocks


def _load_w(nc, pool, src_ap, cols, dtype, tag):
    """DRAM [128, cols] -> SBUF, single DMA (+ cast if bf16)."""
    stage = pool.tile([P, cols], F32, tag=f"{tag}_st")
    nc.sync.dma_start(stage[:], src_ap)
    if dtype == F32:
        return stage
    out = pool.tile([P, cols], dtype, tag=f"{tag}_bf")
    nc.vector.tensor_copy(out[:], stage[:])
    return out


# ---------------------------------------------------------------- program --
def build_program(reps=1):
    nc = bass.Bass()

    x_d = nc.declare_dram_parameter("x", [B_LOC, C, L], F32, isOutput=False)
    # host-precomputed replicated / derived parameters
    W1r_d = nc.declare_dram_parameter("W1r", [NB, P, C], F32, isOutput=False)
    W2b_d = nc.declare_dram_parameter("W2b", [NB, P, P], F32, isOutput=False)
    b1r_d = nc.declare_dram_parameter("b1r", [NB, P], F32, isOutput=False)
    b2r_d = nc.declare_dram_parameter("b2r", [NB, P], F32, isOutput=False)
    out_d = nc.declare_dram_parameter("out", [P], F32, isOutput=True)
    if DEBUG:
        dbg_he = nc.declare_dram_parameter("dbg_he", [P, SLC], BF16, isOutput=True)
        dbg_et = nc.declare_dram_parameter("dbg_et", [P, SLC], BF16, isOutput=True)
        dbg_G = nc.declare_dram_parameter("dbg_G", [P, P], F32, isOutput=True)
        dbg_ksC = nc.declare_dram_parameter("dbg_ksC", [P, 1], F32, isOutput=True)
        dbg_M1 = nc.declare_dram_parameter("dbg_M1", [P, P], BF16, isOutput=True)
        dbg_q = nc.declare_dram_parameter("dbg_q", [P, SLC], BF16, isOutput=True)
        dbg_h1 = nc.declare_dram_parameter("dbg_h1", [P, 2048], BF16, isOutput=True)

    with ExitStack() as ctx:
        tc = ctx.enter_context(tile.TileContext(nc))
        cst = ctx.enter_context(tc.tile_pool(name="cst", bufs=1))
        hcm = ctx.enter_context(tc.tile_pool(name="hcm", bufs=3))
        hex_ = ctx.enter_context(tc.tile_pool(name="hex", bufs=4))
        etm = ctx.enter_context(tc.tile_pool(name="etm", bufs=3))
        qtm = ctx.enter_context(tc.tile_pool(name="qtm", bufs=2))
        sqp = ctx.enter_context(tc.tile_pool(name="sqp", bufs=3))
        bigq = ctx.enter_context(tc.tile_pool(name="bigq", bufs=2))
        a1p = ctx.enter_context(tc.tile_pool(name="a1p", bufs=3))
        smal = ctx.enter_context(tc.tile_pool(name="smal", bufs=2))
        m1p = ctx.enter_context(tc.tile_pool(name="m1p", bufs=2))
        gps = ctx.enter_context(tc.tile_pool(name="gps", bufs=1, space="PSUM"))
        zps = ctx.enter_context(tc.tile_pool(name="zps", bufs=2, space="PSUM"))
        tps = ctx.enter_context(tc.tile_pool(name="tps", bufs=1, space="PSUM"))

        for _rep in range(reps):
            x_cm = x_d[:].rearrange("b c l -> (b c) l")

            def tview(ap):
                return ap.rearrange("p (c l) -> p c l", l=128)


            # ---- constants (single DMAs on sync) -----------------------
            if _rep == 0:
                ident = cst.tile([P, P], BF16)
                make_identity(nc, ident[:])
                ones_bf = cst.tile([P, 1], BF16)
                nc.vector.memset(ones_bf[:], 1.0)
                headmask = cst.tile([P, P], BF16)
                nc.vector.memset(headmask[:], 1.0)
                hm_v = headmask[:].rearrange("p (g i) -> p g i", i=DH)
                nc.gpsimd.affine_select(
                    out=hm_v, in_=hm_v, pattern=[[-DH, P // DH], [0, DH]],
                    compare_op=mybir.AluOpType.is_ge, fill=0.0,
                    base=0, channel_multiplier=1)
                nc.gpsimd.affine_select(
                    out=hm_v, in_=hm_v, pattern=[[DH, P // DH], [0, DH]],
                    compare_op=mybir.AluOpType.is_ge, fill=0.0,
                    base=DH - 1, channel_multiplier=-1)

                W1rep = [_load_w(nc, cst, W1r_d[i], C, BF16, f"w1_{i}")
                         for i in range(NB)]
                W2blk = [_load_w(nc, cst, W2b_d[i], P, BF16, f"w2_{i}")
                         for i in range(NB)]
                b1rep = [_load_w(nc, cst, b1r_d[i].unsqueeze(-1), 1, F32,
                                 f"b1_{i}") for i in range(NB)]
                b2rep = [_load_w(nc, cst, b2r_d[i].unsqueeze(-1), 1, F32,
                                 f"b2_{i}") for i in range(NB)]

            pooled_parts = cst.tile([P, NZ], F32, tag="pool")

            # ---- block-0 x ingest first: casting DMAs + transposes occupy
            # the DMA engines immediately; weight loads follow behind.
            he_tiles = []
            for s in range(NSL):
                he = hex_.tile([P, SLC], BF16, tag="hex")
                xb = hcm.tile([P, SLC], BF16, tag="xb")
                nc.gpsimd.dma_start(xb[:], x_cm[:, SLC * s:SLC * (s + 1)])
                nc.sync.dma_start_transpose(out=tview(he[:]), in_=xb[:])
                he_tiles.append(he)

            def stage_b_half(blk, s, hf, he, ctx_t, G_ps, ks_ps, q_cm,
                             anchor=None):
                """exp -> gram/ksum -> q scale -> q transpose for one half."""
                et, qt, sq, rq = ctx_t
                es = slice(SLC // 2 * hf, SLC // 2 * (hf + 1))
                qs = slice(HCH * 16 * hf, HCH * 16 * (hf + 1))
                ei = nc.scalar.activation(et[:, es], he[:, es], AF.Exp)
                if anchor is not None:
                    add_dep_helper(ei.ins, anchor.ins, sync=True,
                                   reason="cluster exp after gelu")
                for c in range(HCH * hf, HCH * (hf + 1)):
                    ch = slice(128 * c, 128 * (c + 1))
                    st = (s == 0 and c == 0)
                    sp = (s == NSL - 1 and c == NCH - 1)
                    nc.tensor.matmul(
                        G_ps[:], et[:, ch], he[:, ch], start=st, stop=sp)
                    nc.tensor.matmul(
                        ks_ps[:], et[:, ch], ones_bf[:], start=st, stop=sp)
                nc.vector.reduce_sum(
                    sq[:, qs],
                    et[:, es].rearrange("p (c g d) -> p c g d", g=16, d=DH),
                    axis=mybir.AxisListType.X,
                )
                nc.vector.reciprocal(rq[:, qs], sq[:, qs])
                nc.gpsimd.tensor_tensor(
                    qt[:, es].rearrange("p (c g d) -> p c g d", g=16, d=DH),
                    et[:, es].rearrange("p (c g d) -> p c g d", g=16, d=DH),
                    rq[:, qs].rearrange("p (c g) -> p c g", g=16)
                        .unsqueeze(-1).broadcast_to([P, HCH, 16, DH]),
                    op=MUL,
                )
                nc.sync.dma_start_transpose(
                    out=tview(q_cm[:, SLC * s + SLC // 2 * hf:
                                   SLC * s + SLC // 2 * (hf + 1)]),
                    in_=qt[:, es],
                )
                if DEBUG and blk == DBG_BLK and s == 0 and hf == 1:
                    nc.sync.dma_start(dbg_he[:], he[:])
                    nc.sync.dma_start(dbg_et[:], et[:])
                return ei

            def new_slice_ctx():
                et = etm.tile([P, SLC], BF16, tag="etm")
                qt = qtm.tile([P, SLC], BF16, tag="qtm")
                sq = sqp.tile([P, NCH * 16], F32, tag="sq")
                rq = sqp.tile([P, NCH * 16], F32, tag="rq")
                return (et, qt, sq, rq)

            def stage_b(blk, s, he, G_ps, ks_ps, q_cm, anchor=None):
                ctx_t = new_slice_ctx()
                return [stage_b_half(blk, s, hf, he, ctx_t, G_ps, ks_ps,
                                     q_cm, anchor) for hf in range(2)]

            def m1_build(blk, G_ps, ks_ps):
                ksC = smal.tile([P, 1], F32, tag="ksC")
                nc.vector.reciprocal(ksC[:], ks_ps[:])
                G_sb = smal.tile([P, P], BF16, tag="Gsb")
                nc.vector.tensor_tensor(G_sb[:], G_ps[:], headmask[:],
                                        op=MUL)
                GT2_ps = tps.tile([P, C], BF16, tag="tiny")
                for b in range(B_LOC):
                    sl = slice(C * b, C * (b + 1))
                    nc.tensor.transpose(
                        GT2_ps[sl, 0:C], G_sb[sl, sl], ident[sl, sl],
                        tile_position=(C * b, C * b),
                    )
                GT2_sb = smal.tile([P, C], BF16, tag="gt2sb")
                nc.vector.tensor_copy(GT2_sb[:], GT2_ps[:])
                M1u_ps = tps.tile([P, C], F32, tag="tiny")
                for b in range(B_LOC):
                    sl = slice(C * b, C * (b + 1))
                    nc.tensor.matmul(
                        M1u_ps[sl, 0:C], GT2_sb[sl, :], W1rep[blk][sl, :],
                        tile_position=(C * b, C * b),
                    )
                M1blk = m1p.tile([P, P], BF16, tag="m1b")
                nc.gpsimd.memset(M1blk[:], 0.0)
                for b in range(B_LOC):
                    sl = slice(C * b, C * (b + 1))
                    nc.vector.tensor_scalar_mul(
                        M1blk[sl, C * b:C * (b + 1)], M1u_ps[sl, 0:C],
                        ksC[sl, :])
                if DEBUG and blk == DBG_BLK:
                    gtmp = smal.tile([P, P], F32, tag="gdump")
                    nc.vector.tensor_copy(gtmp[:], G_ps[:])
                    nc.sync.dma_start(dbg_G[:], gtmp[:])
                    nc.sync.dma_start(dbg_ksC[:], ksC[:])
                    nc.sync.dma_start(dbg_M1[:], M1blk[:])
                return M1blk

            # ================= block pipeline =========================
            # stage_b(blk) for blocks 1,2 is issued inside block blk-1's
            # phase 2; block 0 runs a stage-parallel standalone phase 1.
            G_ps = gps.tile([P, P], F32, tag="G")
            ks_ps = gps.tile([P, 1], F32, tag="ks")
            q_cm = bigq.tile([P, L], BF16, tag="qcm")
            for s in range(NSL):
                stage_b(0, s, he_tiles[s], G_ps, ks_ps, q_cm)

            for blk in range(NB):
                M1blk = m1_build(blk, G_ps, ks_ps)
                last = blk == NB - 1
                if not last:
                    G_next = gps.tile([P, P], F32, tag="G")
                    ks_next = gps.tile([P, 1], F32, tag="ks")
                    q_next = bigq.tile([P, L], BF16, tag="qcm")
                    he_next = []
                    for _s in range(NSL):
                        hx = hex_.tile([P, SLC], BF16, tag="hex")
                        he_next.append(hx)
                # phase 2 (channel-major); next block's phase 1 interleaved
                pending_exps = []
                for t in range(NZ):
                    z1 = zps.tile([P, ZW], F32, tag="z")
                    for hw in range(2):
                        cs = ZW * t + 512 * hw
                        nc.tensor.matmul(
                            z1[:, 512 * hw:512 * (hw + 1)], M1blk[:],
                            q_cm[:, cs:cs + 512],
                        )
                    a1 = a1p.tile([P, ZW], BF16, tag="a1")
                    a1g = nc.scalar.activation(a1[:], z1[:], AF.Gelu,
                                               bias=b1rep[blk][:])
                    for e in pending_exps:
                        add_dep_helper(a1g.ins, e.ins, sync=True,
                                       reason="cluster exp before gelu")
                    pending_exps = []
                    z2 = zps.tile([P, ZW], F32, tag="z")
                    for hw in range(2):
                        nc.tensor.matmul(
                            z2[:, 512 * hw:512 * (hw + 1)], W2blk[blk][:],
                            a1[:, 512 * hw:512 * (hw + 1)],
                        )
                    if t % 2 == 0:
                        hn = hcm.tile([P, 2048], BF16, tag="hcm")
                    ho = hn[:, ZW * (t % 2):ZW * (t % 2 + 1)]
                    if last:
                        g2 = nc.scalar.activation(
                            ho, z2[:], AF.Gelu, bias=b2rep[blk][:],
                            accum_out=pooled_parts[:, t:t + 1],
                        )
                    else:
                        g2 = nc.scalar.activation(
                            ho, z2[:], AF.Gelu, bias=b2rep[blk][:],
                        )
                    if DEBUG and blk == 0 and t == 0:
                        nc.sync.dma_start(dbg_q[:], q_cm[:, 0:SLC])
                    if DEBUG and blk == 0 and t == 1:
                        nc.sync.dma_start(dbg_h1[:], hn[:])
                    if t % 2 == 1 and not last:
                        # next block's token-major h via DMA transpose
                        s = t // 4
                        hh = (t // 2) % 2
                        nc.sync.dma_start_transpose(
                            out=tview(he_next[s][:, 2048 * hh:
                                                 2048 * (hh + 1)]),
                            in_=hn[:],
                        )
                    # grouped next-block phase-1 issue
                    if not last:
                        if t == 7:
                            pending_exps += stage_b(
                                blk + 1, 0, he_next[0], G_next, ks_next,
                                q_next, anchor=g2)
                            pending_exps += stage_b(
                                blk + 1, 1, he_next[1], G_next, ks_next,
                                q_next, anchor=g2)
                        elif t == 11:
                            pending_exps += stage_b(
                                blk + 1, 2, he_next[2], G_next, ks_next,
                                q_next, anchor=g2)
                        elif t == 13:
                            s3ctx = new_slice_ctx()
                            pending_exps.append(stage_b_half(
                                blk + 1, 3, 0, he_next[3], s3ctx, G_next,
                                ks_next, q_next, anchor=g2))
                        elif t == 15:
                            stage_b_half(blk + 1, 3, 1, he_next[3], s3ctx,
                                         G_next, ks_next, q_next, anchor=g2)
                if not last:
                    G_ps, ks_ps, q_cm = G_next, ks_next, q_next

            # ===================== head (host-side finish) ==========
            psum_ = smal.tile([P, 1], F32, tag="poolsum")
            nc.vector.reduce_sum(psum_[:], pooled_parts[:],
                                 axis=mybir.AxisListType.X)
            nc.sync.dma_start(out_d[:], psum_[:, 0])

    _fix_sync_waits(nc)
    return nc


def _host_head(psum, arrs):
    """Finish the network head on the host: pooled mean -> dense -> BN ->
    gelu -> dense. psum is the per-core pooled SUM [128] = (4b x 32c)."""
    import math
    pooled = psum.reshape(B_LOC, C) / L
    y = pooled @ arrs["Wh"] + arrs["bh"]
    svec = arrs["bn_gamma"] / np.sqrt(arrs["bn_var"] + BN_EPS)
    y = (y - arrs["bn_mean"]) * svec + arrs["bn_beta"]
    erf = np.vectorize(math.erf)
    y = 0.5 * y * (1.0 + erf(y / math.sqrt(2.0)))
    return y @ arrs["Wf"] + arrs["bf"]


def _derive_params(arrs):
    """Host-side precompute: replicated weight strips + BN eval folding."""
    tile4 = lambda a: np.tile(a, (B_LOC,) + (1,) * (a.ndim - 1))
    W1 = arrs["fcW1"]            # [NB, C, C]
    W2 = arrs["fcW2"]
    W2b = np.zeros((NB, P, P), np.float32)
    for i in range(NB):
        for b in range(B_LOC):
            W2b[i, C * b:C * (b + 1), C * b:C * (b + 1)] = W2[i]
    return {
        "W1r": np.ascontiguousarray(
            np.stack([tile4(W1[i]) for i in range(NB)])),
        "W2b": W2b,
        "b1r": np.ascontiguousarray(
            np.stack([tile4(arrs["fcb1"][i]) for i in range(NB)])),
        "b2r": np.ascontiguousarray(
            np.stack([tile4(arrs["fcb2"][i]) for i in range(NB)])),
    }


_NC_CACHE = [None]


def kernel(**inputs) -> np.ndarray:
    arrs = {k: np.asarray(v, dtype=np.float32) for k, v in inputs.items()}
    x = arrs["x"]
    B = x.shape[0]
    n_cores = 8
    bl = B // n_cores

    if _NC_CACHE[0] is None:
        _NC_CACHE[0] = build_program()
    nc = _NC_CACHE[0]

    params = _derive_params(arrs)
    in_maps = [
        {"x": np.ascontiguousarray(x[bl * i: bl * (i + 1)]), **params}
        for i in range(n_cores)
    ]
    res = run_bass_kernel_spmd(nc, in_maps, list(range(n_cores))).results
    return np.concatenate(
        [_host_head(res[i]["out"], arrs) for i in range(n_cores)], axis=0)


# revision 30
# speedup vs baseline: 1.0775x; 1.0775x over previous
"""Trainium2 Bass kernel for nn_CLFormer (3-block linear-attention transformer).

Sharding: pure data parallel — batch 32 split as 4 per NeuronCore across 8
cores; all parameters replicated; outputs concatenated.

Per-core layout: 4 batches x 32 channels packed on the 128 SBUF partitions
("channel-major" [128=4bx32c, L]). The kv-gram contracts over tokens, so a
token-major copy is produced per block by SBUF->SBUF DMA transposes (block
0 from a bf16 cast of x loaded via casting DMA; blocks 1-2 from the gelu
output during the previous phase 2 — no DRAM round trip, no PE transposes).
q returns to channel-major the same way. The k-softmax denominator comes
from a 1-moving-row matmul (et stationary x ones) accumulated next to the
gram. Attention-out and FC1 fuse into M1 = diag(1/ksum) @ G @ W1; FC
matmuls run as single 128-wide matmuls with block-diagonal weights. Phase 1
of block b+1 is issued inside block b's phase 2 so all engines stay busy
across block boundaries. Replicated/derived parameters (weight strips, BN
eval folding) are precomputed on the host.
"""
import sys
import numpy as np

for _p in ("/opt/trn_rl_repo", "/root/.axon_site/_ro/trn_rl_repo"):
    if _p not in sys.path:
        sys.path.append(_p)

from contextlib import ExitStack

import concourse.bass as bass
import concourse.mybir as mybir
import bass_rust
from bass_rust import add_dep_helper
from concourse import tile
from concourse.masks import make_identity
from concourse.bass_utils import run_bass_kernel_spmd

F32 = mybir.dt.float32
BF16 = mybir.dt.bfloat16
AF = mybir.ActivationFunctionType
MUL = mybir.AluOpType.mult
ADD = mybir.AluOpType.add

P = 128
B_LOC = 4            # batches per core
C = 32               # channels
L = 16384            # sequence length
NB = 3               # transformer blocks
DOUT = 10
HEADS = 4
DH = 8
BN_EPS = 1e-5

SLC = 4096           # slice width (tokens per phase-1 slice)
NSL = L // SLC       # 4 slices
NCH = SLC // 128     # 32 chunks per slice
HCH = NCH // 2       # chunks per half slice
ZW = 1024            # phase-2 tile width (tokens per z/gelu tile)
NZ = L // ZW         # 16 phase-2 steps
DEBUG = False
DBG_BLK = 0


# ---------------------------------------------------------------- waitfix --
_WF_SKIP = {"InstEventSemaphore"}
_wf_ctr = [0]


def _fix_sync_waits(nc):
    """Hoist excess sync waits onto InstEventSemaphore (this walrus build
    accepts only 1 wait per instruction). The event-sem executes on the same
    engine stream immediately before, preserving semantics."""
    for fn in nc.m.functions:
        new_blocks = []
        for blk in fn.blocks:
            out = []
            for ins in blk.instructions:
                tname = type(ins).__name__
                si = ins.sync_info
                if si is None or tname in _WF_SKIP:
                    out.append(ins)
                    continue
                waits = list(si.on_wait)
                if len(waits) <= 1:
                    out.append(ins)
                    continue
                keep = waits[-1:]
                excess = waits[:-1]
                for i in range(0, len(excess), 2):
                    chunk = excess[i:i + 2]
                    _wf_ctr[0] += 1
                    ev = mybir.InstEventSemaphore(
                        name=f"wfix{_wf_ctr[0]}", ins=[], outs=[])
                    ev.engine = ins.engine
                    ev.sync_info = mybir.SyncInfo(on_wait=chunk, on_update=[])
                    out.append(ev)
                ins.sync_info = mybir.SyncInfo(
                    on_wait=keep, on_update=list(si.on_update))
                out.append(ins)
            nb = bass_rust.BasicBlock(name=blk.name, instructions=out)
            new_blocks.append(nb)
        fn.blocks = new_blocks


def _load_w(nc, pool, src_ap, cols, dtype, tag):
    """DRAM [128, cols] -> SBUF, single DMA (+ cast if bf16)."""
    stage = pool.tile([P, cols], F32, tag=f"{tag}_st")
    nc.sync.dma_start(stage[:], src_ap)
    if dtype == F32:
        return stage
    out = pool.tile([P, cols], dtype, tag=f"{tag}_bf")
    nc.vector.tensor_copy(out[:], stage[:])
    return out


# ---------------------------------------------------------------- program --
def build_program(reps=1):
    nc = bass.Bass()

    x_d = nc.declare_dram_parameter("x", [B_LOC, C, L], F32, isOutput=False)
    # host-precomputed replicated / derived parameters
    W1r_d = nc.declare_dram_parameter("W1r", [NB, P, C], F32, isOutput=False)
    W2b_d = nc.declare_dram_parameter("W2b", [NB, P, P], F32, isOutput=False)
    b1r_d = nc.declare_dram_parameter("b1r", [NB, P], F32, isOutput=False)
    b2r_d = nc.declare_dram_parameter("b2r", [NB, P], F32, isOutput=False)
    out_d = nc.declare_dram_parameter("out", [P], F32, isOutput=True)
    if DEBUG:
        dbg_he = nc.declare_dram_parameter("dbg_he", [P, SLC], BF16, isOutput=True)
        dbg_et = nc.declare_dram_parameter("dbg_et", [P, SLC], BF16, isOutput=True)
        dbg_G = nc.declare_dram_parameter("dbg_G", [P, P], F32, isOutput=True)
        dbg_ksC = nc.declare_dram_parameter("dbg_ksC", [P, 1], F32, isOutput=True)
        dbg_M1 = nc.declare_dram_parameter("dbg_M1", [P, P], BF16, isOutput=True)
        dbg_q = nc.declare_dram_parameter("dbg_q", [P, SLC], BF16, isOutput=True)
        dbg_h1 = nc.declare_dram_parameter("dbg_h1", [P, 2048], BF16, isOutput=True)

    with ExitStack() as ctx:
        tc = ctx.enter_context(tile.TileContext(nc))
        cst = ctx.enter_context(tc.tile_pool(name="cst", bufs=1))
        hcm = ctx.enter_context(tc.tile_pool(name="hcm", bufs=3))
        hex_ = ctx.enter_context(tc.tile_pool(name="hex", bufs=4))
        etm = ctx.enter_context(tc.tile_pool(name="etm", bufs=3))
        qtm = ctx.enter_context(tc.tile_pool(name="qtm", bufs=2))
        sqp = ctx.enter_context(tc.tile_pool(name="sqp", bufs=3))
        bigq = ctx.enter_context(tc.tile_pool(name="bigq", bufs=2))
        a1p = ctx.enter_context(tc.tile_pool(name="a1p", bufs=3))
        smal = ctx.enter_context(tc.tile_pool(name="smal", bufs=2))
        m1p = ctx.enter_context(tc.tile_pool(name="m1p", bufs=2))
        gps = ctx.enter_context(tc.tile_pool(name="gps", bufs=1, space="PSUM"))
        zps = ctx.enter_context(tc.tile_pool(name="zps", bufs=2, space="PSUM"))
        tps = ctx.enter_context(tc.tile_pool(name="tps", bufs=1, space="PSUM"))

        for _rep in range(reps):
            x_cm = x_d[:].rearrange("b c l -> (b c) l")

            def tview(ap):
                return ap.rearrange("p (c l) -> p c l", l=128)


            # ---- constants (single DMAs on sync) -----------------------
            if _rep == 0:
                ident = cst.tile([P, P], BF16)
                make_identity(nc, ident[:])
                ones_bf = cst.tile([P, 1], BF16)
                nc.vector.memset(ones_bf[:], 1.0)
                headmask = cst.tile([P, P], BF16)
                nc.vector.memset(headmask[:], 1.0)
                hm_v = headmask[:].rearrange("p (g i) -> p g i", i=DH)
                nc.gpsimd.affine_select(
                    out=hm_v, in_=hm_v, pattern=[[-DH, P // DH], [0, DH]],
                    compare_op=mybir.AluOpType.is_ge, fill=0.0,
                    base=0, channel_multiplier=1)
                nc.gpsimd.affine_select(
                    out=hm_v, in_=hm_v, pattern=[[DH, P // DH], [0, DH]],
                    compare_op=mybir.AluOpType.is_ge, fill=0.0,
                    base=DH - 1, channel_multiplier=-1)

                W1rep = [_load_w(nc, cst, W1r_d[i], C, BF16, f"w1_{i}")
                         for i in range(NB)]
                W2blk = [_load_w(nc, cst, W2b_d[i], P, BF16, f"w2_{i}")
                         for i in range(NB)]
                b1rep = [_load_w(nc, cst, b1r_d[i].unsqueeze(-1), 1, F32,
                                 f"b1_{i}") for i in range(NB)]
                b2rep = [_load_w(nc, cst, b2r_d[i].unsqueeze(-1), 1, F32,
                                 f"b2_{i}") for i in range(NB)]

            pooled_parts = cst.tile([P, NZ], F32, tag="pool")

            # ---- block-0 x ingest first: casting DMAs + transposes occupy
            # the DMA engines immediately; weight loads follow behind.
            he_tiles = []
            for s in range(NSL):
                he = hex_.tile([P, SLC], BF16, tag="hex")
                xb = hcm.tile([P, SLC], BF16, tag="xb")
                nc.gpsimd.dma_start(xb[:], x_cm[:, SLC * s:SLC * (s + 1)])
                nc.sync.dma_start_transpose(out=tview(he[:]), in_=xb[:])
                he_tiles.append(he)

            def stage_b_half(blk, s, hf, he, ctx_t, G_ps, ks_ps, q_cm,
                             anchor=None):
                """exp -> gram/ksum -> q scale -> q transpose for one half."""
                et, qt, sq, rq = ctx_t
                es = slice(SLC // 2 * hf, SLC // 2 * (hf + 1))
                qs = slice(HCH * 16 * hf, HCH * 16 * (hf + 1))
                ei = nc.scalar.activation(et[:, es], he[:, es], AF.Exp)
                if anchor is not None:
                    add_dep_helper(ei.ins, anchor.ins, sync=True,
                                   reason="cluster exp after gelu")
                for c in range(HCH * hf, HCH * (hf + 1)):
                    ch = slice(128 * c, 128 * (c + 1))
                    st = (s == 0 and c == 0)
                    sp = (s == NSL - 1 and c == NCH - 1)
                    nc.tensor.matmul(
                        G_ps[:], et[:, ch], he[:, ch], start=st, stop=sp)
                    nc.tensor.matmul(
                        ks_ps[:], et[:, ch], ones_bf[:], start=st, stop=sp)
                nc.vector.reduce_sum(
                    sq[:, qs],
                    et[:, es].rearrange("p (c g d) -> p c g d", g=16, d=DH),
                    axis=mybir.AxisListType.X,
                )
                nc.vector.reciprocal(rq[:, qs], sq[:, qs])
                nc.gpsimd.tensor_tensor(
                    qt[:, es].rearrange("p (c g d) -> p c g d", g=16, d=DH),
                    et[:, es].rearrange("p (c g d) -> p c g d", g=16, d=DH),
                    rq[:, qs].rearrange("p (c g) -> p c g", g=16)
                        .unsqueeze(-1).broadcast_to([P, HCH, 16, DH]),
                    op=MUL,
                )
                nc.sync.dma_start_transpose(
                    out=tview(q_cm[:, SLC * s + SLC // 2 * hf:
                                   SLC * s + SLC // 2 * (hf + 1)]),
                    in_=qt[:, es],
                )
                if DEBUG and blk == DBG_BLK and s == 0 and hf == 1:
                    nc.sync.dma_start(dbg_he[:], he[:])
                    nc.sync.dma_start(dbg_et[:], et[:])
                return ei

            def new_slice_ctx():
                et = etm.tile([P, SLC], BF16, tag="etm")
                qt = qtm.tile([P, SLC], BF16, tag="qtm")
                sq = sqp.tile([P, NCH * 16], F32, tag="sq")
                rq = sqp.tile([P, NCH * 16], F32, tag="rq")
                return (et, qt, sq, rq)

            def stage_b(blk, s, he, G_ps, ks_ps, q_cm, anchor=None):
                ctx_t = new_slice_ctx()
                return [stage_b_half(blk, s, hf, he, ctx_t, G_ps, ks_ps,
                                     q_cm, anchor) for hf in range(2)]

            def m1_build(blk, G_ps, ks_ps):
                ksC = smal.tile([P, 1], F32, tag="ksC")
                nc.vector.reciprocal(ksC[:], ks_ps[:])
                G_sb = smal.tile([P, P], BF16, tag="Gsb")
                nc.vector.tensor_tensor(G_sb[:], G_ps[:], headmask[:],
                                        op=MUL)
                GT2_ps = tps.tile([P, C], BF16, tag="tiny")
                for b in range(B_LOC):
                    sl = slice(C * b, C * (b + 1))
                    nc.tensor.transpose(
                        GT2_ps[sl, 0:C], G_sb[sl, sl], ident[sl, sl],
                        tile_position=(C * b, C * b),
                    )
                GT2_sb = smal.tile([P, C], BF16, tag="gt2sb")
                nc.vector.tensor_copy(GT2_sb[:], GT2_ps[:])
                M1u_ps = tps.tile([P, C], F32, tag="tiny")
                for b in range(B_LOC):
                    sl = slice(C * b, C * (b + 1))
                    nc.tensor.matmul(
                        M1u_ps[sl, 0:C], GT2_sb[sl, :], W1rep[blk][sl, :],
                        tile_position=(C * b, C * b),
                    )
                M1blk = m1p.tile([P, P], BF16, tag="m1b")
                nc.gpsimd.memset(M1blk[:], 0.0)
                for b in range(B_LOC):
                    sl = slice(C * b, C * (b + 1))
                    nc.vector.tensor_scalar_mul(
                        M1blk[sl, C * b:C * (b + 1)], M1u_ps[sl, 0:C],
                        ksC[sl, :])
                if DEBUG and blk == DBG_BLK:
                    gtmp = smal.tile([P, P], F32, tag="gdump")
                    nc.vector.tensor_copy(gtmp[:], G_ps[:])
                    nc.sync.dma_start(dbg_G[:], gtmp[:])
                    nc.sync.dma_start(dbg_ksC[:], ksC[:])
                    nc.sync.dma_start(dbg_M1[:], M1blk[:])
                return M1blk

            # ================= block pipeline =========================
            # stage_b(blk) for blocks 1,2 is issued inside block blk-1's
            # phase 2; block 0 runs a stage-parallel standalone phase 1.
            G_ps = gps.tile([P, P], F32, tag="G")
            ks_ps = gps.tile([P, 1], F32, tag="ks")
            q_cm = bigq.tile([P, L], BF16, tag="qcm")
            for s in range(NSL):
                stage_b(0, s, he_tiles[s], G_ps, ks_ps, q_cm)

            for blk in range(NB):
                M1blk = m1_build(blk, G_ps, ks_ps)
                last = blk == NB - 1
                if not last:
                    G_next = gps.tile([P, P], F32, tag="G")
                    ks_next = gps.tile([P, 1], F32, tag="ks")
                    q_next = bigq.tile([P, L], BF16, tag="qcm")
                    he_next = []
                    for _s in range(NSL):
                        hx = hex_.tile([P, SLC], BF16, tag="hex")
                        he_next.append(hx)
                # phase 2 (channel-major); next block's phase 1 interleaved
                pending_exps = []
                for t in range(NZ):
                    z1 = zps.tile([P, ZW], F32, tag="z")
                    for hw in range(2):
                        cs = ZW * t + 512 * hw
                        nc.tensor.matmul(
                            z1[:, 512 * hw:512 * (hw + 1)], M1blk[:],
                            q_cm[:, cs:cs + 512],
                        )
                    a1 = a1p.tile([P, ZW], BF16, tag="a1")
                    a1g = nc.scalar.activation(a1[:], z1[:], AF.Gelu,
                                               bias=b1rep[blk][:])
                    for e in pending_exps:
                        add_dep_helper(a1g.ins, e.ins, sync=True,
                                       reason="cluster exp before gelu")
                    pending_exps = []
                    z2 = zps.tile([P, ZW], F32, tag="z")
                    for hw in range(2):
                        nc.tensor.matmul(
                            z2[:, 512 * hw:512 * (hw + 1)], W2blk[blk][:],
                            a1[:, 512 * hw:512 * (hw + 1)],
                        )
                    if t % 2 == 0:
                        hn = hcm.tile([P, 2048], BF16, tag="hcm")
                    ho = hn[:, ZW * (t % 2):ZW * (t % 2 + 1)]
                    if last:
                        g2 = nc.scalar.activation(
                            ho, z2[:], AF.Gelu, bias=b2rep[blk][:],
                            accum_out=pooled_parts[:, t:t + 1],
                        )
                    else:
                        g2 = nc.scalar.activation(
                            ho, z2[:], AF.Gelu, bias=b2rep[blk][:],
                        )
                    if DEBUG and blk == 0 and t == 0:
                        nc.sync.dma_start(dbg_q[:], q_cm[:, 0:SLC])
                    if DEBUG and blk == 0 and t == 1:
                        nc.sync.dma_start(dbg_h1[:], hn[:])
                    if t % 2 == 1 and not last:
                        # next block's token-major h via DMA transpose
                        s = t // 4
                        hh = (t // 2) % 2
                        nc.sync.dma_start_transpose(
                            out=tview(he_next[s][:, 2048 * hh:
                                                 2048 * (hh + 1)]),
                            in_=hn[:],
                        )
                    # grouped next-block phase-1 issue
                    if not last:
                        if t == 7:
                            pending_exps += stage_b(
                                blk + 1, 0, he_next[0], G_next, ks_next,
                                q_next, anchor=g2)
                            pending_exps += stage_b(
                                blk + 1, 1, he_next[1], G_next, ks_next,
                                q_next, anchor=g2)
                        elif t == 11:
                            pending_exps += stage_b(
                                blk + 1, 2, he_next[2], G_next, ks_next,
                                q_next, anchor=g2)
                        elif t == 15:
                            stage_b(blk + 1, 3, he_next[3], G_next, ks_next,
                                    q_next, anchor=g2)
                if not last:
                    G_ps, ks_ps, q_cm = G_next, ks_next, q_next

            # ===================== head (host-side finish) ==========
            psum_ = smal.tile([P, 1], F32, tag="poolsum")
            nc.vector.reduce_sum(psum_[:], pooled_parts[:],
                                 axis=mybir.AxisListType.X)
            nc.sync.dma_start(out_d[:], psum_[:, 0])

    _fix_sync_waits(nc)
    return nc


def _host_head(psum, arrs):
    """Finish the network head on the host: pooled mean -> dense -> BN ->
    gelu -> dense. psum is the per-core pooled SUM [128] = (4b x 32c)."""
    import math
    pooled = psum.reshape(B_LOC, C) / L
    y = pooled @ arrs["Wh"] + arrs["bh"]
    svec = arrs["bn_gamma"] / np.sqrt(arrs["bn_var"] + BN_EPS)
    y = (y - arrs["bn_mean"]) * svec + arrs["bn_beta"]
    erf = np.vectorize(math.erf)
    y = 0.5 * y * (1.0 + erf(y / math.sqrt(2.0)))
    return y @ arrs["Wf"] + arrs["bf"]


def _derive_params(arrs):
    """Host-side precompute: replicated weight strips + BN eval folding."""
    tile4 = lambda a: np.tile(a, (B_LOC,) + (1,) * (a.ndim - 1))
    W1 = arrs["fcW1"]            # [NB, C, C]
    W2 = arrs["fcW2"]
    W2b = np.zeros((NB, P, P), np.float32)
    for i in range(NB):
        for b in range(B_LOC):
            W2b[i, C * b:C * (b + 1), C * b:C * (b + 1)] = W2[i]
    return {
        "W1r": np.ascontiguousarray(
            np.stack([tile4(W1[i]) for i in range(NB)])),
        "W2b": W2b,
        "b1r": np.ascontiguousarray(
            np.stack([tile4(arrs["fcb1"][i]) for i in range(NB)])),
        "b2r": np.ascontiguousarray(
            np.stack([tile4(arrs["fcb2"][i]) for i in range(NB)])),
    }


_NC_CACHE = [None]


def kernel(**inputs) -> np.ndarray:
    arrs = {k: np.asarray(v, dtype=np.float32) for k, v in inputs.items()}
    x = arrs["x"]
    B = x.shape[0]
    n_cores = 8
    bl = B // n_cores

    if _NC_CACHE[0] is None:
        _NC_CACHE[0] = build_program()
    nc = _NC_CACHE[0]

    params = _derive_params(arrs)
    in_maps = [
        {"x": np.ascontiguousarray(x[bl * i: bl * (i + 1)]), **params}
        for i in range(n_cores)
    ]
    res = run_bass_kernel_spmd(nc, in_maps, list(range(n_cores))).results
    return np.concatenate(
        [_host_head(res[i]["out"], arrs) for i in range(n_cores)], axis=0)


# revision 31
# speedup vs baseline: 1.1225x; 1.0418x over previous
"""Trainium2 Bass kernel for nn_CLFormer (3-block linear-attention transformer).

Sharding: pure data parallel — batch 32 split as 4 per NeuronCore across 8
cores; all parameters replicated; outputs concatenated.

Per-core layout: 4 batches x 32 channels packed on the 128 SBUF partitions
("channel-major" [128=4bx32c, L]). The kv-gram contracts over tokens, so a
token-major copy is produced per block by SBUF->SBUF DMA transposes (block
0 from a bf16 cast of x loaded via casting DMA; blocks 1-2 from the gelu
output during the previous phase 2 — no DRAM round trip, no PE transposes).
q returns to channel-major the same way. The k-softmax denominator comes
from a 1-moving-row matmul (et stationary x ones) accumulated next to the
gram. Attention-out and FC1 fuse into M1 = diag(1/ksum) @ G @ W1; FC
matmuls run as single 128-wide matmuls with block-diagonal weights. Phase 1
of block b+1 is issued inside block b's phase 2 so all engines stay busy
across block boundaries. Replicated/derived parameters (weight strips, BN
eval folding) are precomputed on the host.
"""
import sys
import numpy as np

for _p in ("/opt/trn_rl_repo", "/root/.axon_site/_ro/trn_rl_repo"):
    if _p not in sys.path:
        sys.path.append(_p)

from contextlib import ExitStack

import concourse.bass as bass
import concourse.mybir as mybir
import bass_rust
from bass_rust import add_dep_helper
from concourse import tile
from concourse.masks import make_identity
from concourse.bass_utils import run_bass_kernel_spmd

F32 = mybir.dt.float32
BF16 = mybir.dt.bfloat16
AF = mybir.ActivationFunctionType
MUL = mybir.AluOpType.mult
ADD = mybir.AluOpType.add

P = 128
B_LOC = 4            # batches per core
C = 32               # channels
L = 16384            # sequence length
NB = 3               # transformer blocks
DOUT = 10
HEADS = 4
DH = 8
BN_EPS = 1e-5

SLC = 4096           # slice width (tokens per phase-1 slice)
NSL = L // SLC       # 4 slices
NCH = SLC // 128     # 32 chunks per slice
HCH = NCH // 2       # chunks per half slice
ZW = 1024            # phase-2 tile width (tokens per z/gelu tile)
NZ = L // ZW         # 16 phase-2 steps
DEBUG = False
DBG_BLK = 0


# ---------------------------------------------------------------- waitfix --
_WF_SKIP = {"InstEventSemaphore"}
_wf_ctr = [0]


def _fix_sync_waits(nc):
    """Hoist excess sync waits onto InstEventSemaphore (this walrus build
    accepts only 1 wait per instruction). The event-sem executes on the same
    engine stream immediately before, preserving semantics."""
    for fn in nc.m.functions:
        new_blocks = []
        for blk in fn.blocks:
            out = []
            for ins in blk.instructions:
                tname = type(ins).__name__
                si = ins.sync_info
                if si is None or tname in _WF_SKIP:
                    out.append(ins)
                    continue
                waits = list(si.on_wait)
                if len(waits) <= 1:
                    out.append(ins)
                    continue
                keep = waits[-1:]
                excess = waits[:-1]
                for i in range(0, len(excess), 2):
                    chunk = excess[i:i + 2]
                    _wf_ctr[0] += 1
                    ev = mybir.InstEventSemaphore(
                        name=f"wfix{_wf_ctr[0]}", ins=[], outs=[])
                    ev.engine = ins.engine
                    ev.sync_info = mybir.SyncInfo(on_wait=chunk, on_update=[])
                    out.append(ev)
                ins.sync_info = mybir.SyncInfo(
                    on_wait=keep, on_update=list(si.on_update))
                out.append(ins)
            nb = bass_rust.BasicBlock(name=blk.name, instructions=out)
            new_blocks.append(nb)
        fn.blocks = new_blocks


def _load_w(nc, pool, src_ap, cols, dtype, tag):
    """DRAM [128, cols] -> SBUF, single DMA (+ cast if bf16)."""
    stage = pool.tile([P, cols], F32, tag=f"{tag}_st")
    nc.sync.dma_start(stage[:], src_ap)
    if dtype == F32:
        return stage
    out = pool.tile([P, cols], dtype, tag=f"{tag}_bf")
    nc.vector.tensor_copy(out[:], stage[:])
    return out


# ---------------------------------------------------------------- program --
def build_program(reps=1):
    nc = bass.Bass()

    x_d = nc.declare_dram_parameter("x", [B_LOC, C, L], F32, isOutput=False)
    # host-precomputed replicated / derived parameters
    W1r_d = nc.declare_dram_parameter("W1r", [NB, P, C], F32, isOutput=False)
    W2b_d = nc.declare_dram_parameter("W2b", [NB, P, P], F32, isOutput=False)
    b1r_d = nc.declare_dram_parameter("b1r", [NB, P], F32, isOutput=False)
    b2r_d = nc.declare_dram_parameter("b2r", [NB, P], F32, isOutput=False)
    Whr_d = nc.declare_dram_parameter("Whr", [P, C], F32, isOutput=False)
    Wfr_d = nc.declare_dram_parameter("Wfr", [P, DOUT], F32, isOutput=False)
    svecL_d = nc.declare_dram_parameter("svecLr", [P], F32, isOutput=False)
    tvec_d = nc.declare_dram_parameter("tvecr", [P], F32, isOutput=False)
    bf_d = nc.declare_dram_parameter("bfr", [P], F32, isOutput=False)
    out_d = nc.declare_dram_parameter("out", [B_LOC, DOUT], F32, isOutput=True)
    if DEBUG:
        dbg_he = nc.declare_dram_parameter("dbg_he", [P, SLC], BF16, isOutput=True)
        dbg_et = nc.declare_dram_parameter("dbg_et", [P, SLC], BF16, isOutput=True)
        dbg_G = nc.declare_dram_parameter("dbg_G", [P, P], F32, isOutput=True)
        dbg_ksC = nc.declare_dram_parameter("dbg_ksC", [P, 1], F32, isOutput=True)
        dbg_M1 = nc.declare_dram_parameter("dbg_M1", [P, P], BF16, isOutput=True)
        dbg_q = nc.declare_dram_parameter("dbg_q", [P, SLC], BF16, isOutput=True)
        dbg_h1 = nc.declare_dram_parameter("dbg_h1", [P, 2048], BF16, isOutput=True)

    with ExitStack() as ctx:
        tc = ctx.enter_context(tile.TileContext(nc))
        cst = ctx.enter_context(tc.tile_pool(name="cst", bufs=1))
        hcm = ctx.enter_context(tc.tile_pool(name="hcm", bufs=3))
        hex_ = ctx.enter_context(tc.tile_pool(name="hex", bufs=4))
        etm = ctx.enter_context(tc.tile_pool(name="etm", bufs=3))
        qtm = ctx.enter_context(tc.tile_pool(name="qtm", bufs=2))
        sqp = ctx.enter_context(tc.tile_pool(name="sqp", bufs=3))
        bigq = ctx.enter_context(tc.tile_pool(name="bigq", bufs=2))
        a1p = ctx.enter_context(tc.tile_pool(name="a1p", bufs=3))
        smal = ctx.enter_context(tc.tile_pool(name="smal", bufs=2))
        m1p = ctx.enter_context(tc.tile_pool(name="m1p", bufs=2))
        gps = ctx.enter_context(tc.tile_pool(name="gps", bufs=1, space="PSUM"))
        zps = ctx.enter_context(tc.tile_pool(name="zps", bufs=2, space="PSUM"))
        tps = ctx.enter_context(tc.tile_pool(name="tps", bufs=1, space="PSUM"))

        for _rep in range(reps):
            x_cm = x_d[:].rearrange("b c l -> (b c) l")

            def tview(ap):
                return ap.rearrange("p (c l) -> p c l", l=128)


            # ---- constants (single DMAs on sync) -----------------------
            if _rep == 0:
                ident = cst.tile([P, P], BF16)
                make_identity(nc, ident[:])
                ones_bf = cst.tile([P, 1], BF16)
                nc.vector.memset(ones_bf[:], 1.0)
                headmask = cst.tile([P, P], BF16)
                nc.vector.memset(headmask[:], 1.0)
                hm_v = headmask[:].rearrange("p (g i) -> p g i", i=DH)
                nc.gpsimd.affine_select(
                    out=hm_v, in_=hm_v, pattern=[[-DH, P // DH], [0, DH]],
                    compare_op=mybir.AluOpType.is_ge, fill=0.0,
                    base=0, channel_multiplier=1)
                nc.gpsimd.affine_select(
                    out=hm_v, in_=hm_v, pattern=[[DH, P // DH], [0, DH]],
                    compare_op=mybir.AluOpType.is_ge, fill=0.0,
                    base=DH - 1, channel_multiplier=-1)

                W1rep = [_load_w(nc, cst, W1r_d[i], C, BF16, f"w1_{i}")
                         for i in range(NB)]
                W2blk = [_load_w(nc, cst, W2b_d[i], P, BF16, f"w2_{i}")
                         for i in range(NB)]
                b1rep = [_load_w(nc, cst, b1r_d[i].unsqueeze(-1), 1, F32,
                                 f"b1_{i}") for i in range(NB)]
                b2rep = [_load_w(nc, cst, b2r_d[i].unsqueeze(-1), 1, F32,
                                 f"b2_{i}") for i in range(NB)]
                Whrep = _load_w(nc, cst, Whr_d[:], C, F32, "wh")
                Wfrep = _load_w(nc, cst, Wfr_d[:], DOUT, F32, "wf")
                svecL = _load_w(nc, cst, svecL_d[:].unsqueeze(-1), 1, F32, "sv")
                tvec = _load_w(nc, cst, tvec_d[:].unsqueeze(-1), 1, F32, "tv")
                bf_s = _load_w(nc, cst, bf_d[:].unsqueeze(-1), 1, F32, "bf")

            pooled_parts = cst.tile([P, NZ], F32, tag="pool")

            # ---- block-0 x ingest first: casting DMAs + transposes occupy
            # the DMA engines immediately; weight loads follow behind.
            he_tiles = []
            for s in range(NSL):
                he = hex_.tile([P, SLC], BF16, tag="hex")
                xb = hcm.tile([P, SLC], BF16, tag="xb")
                nc.gpsimd.dma_start(xb[:], x_cm[:, SLC * s:SLC * (s + 1)])
                nc.sync.dma_start_transpose(out=tview(he[:]), in_=xb[:])
                he_tiles.append(he)

            def stage_b(blk, s, he, G_ps, ks_ps, q_cm, anchor=None):
                """exp -> gram/ksum -> q scale -> q transpose (half-split)."""
                et = etm.tile([P, SLC], BF16, tag="etm")
                qt = qtm.tile([P, SLC], BF16, tag="qtm")
                sq = sqp.tile([P, NCH * 16], F32, tag="sq")
                rq = sqp.tile([P, NCH * 16], F32, tag="rq")
                exps = []
                for hf in range(2):
                    es = slice(SLC // 2 * hf, SLC // 2 * (hf + 1))
                    qs = slice(HCH * 16 * hf, HCH * 16 * (hf + 1))
                    ei = nc.scalar.activation(et[:, es], he[:, es], AF.Exp)
                    if anchor is not None:
                        add_dep_helper(ei.ins, anchor.ins, sync=True,
                                       reason="cluster exp after gelu")
                    exps.append(ei)
                    for c in range(HCH * hf, HCH * (hf + 1)):
                        ch = slice(128 * c, 128 * (c + 1))
                        st = (s == 0 and c == 0)
                        sp = (s == NSL - 1 and c == NCH - 1)
                        nc.tensor.matmul(
                            G_ps[:], et[:, ch], he[:, ch],
                            start=st, stop=sp,
                        )
                        nc.tensor.matmul(
                            ks_ps[:], et[:, ch], ones_bf[:],
                            start=st, stop=sp,
                        )
                    nc.vector.reduce_sum(
                        sq[:, qs],
                        et[:, es].rearrange("p (c g d) -> p c g d", g=16, d=DH),
                        axis=mybir.AxisListType.X,
                    )
                    nc.vector.reciprocal(rq[:, qs], sq[:, qs])
                    nc.gpsimd.tensor_tensor(
                        qt[:, es].rearrange("p (c g d) -> p c g d", g=16, d=DH),
                        et[:, es].rearrange("p (c g d) -> p c g d", g=16, d=DH),
                        rq[:, qs].rearrange("p (c g) -> p c g", g=16)
                            .unsqueeze(-1).broadcast_to([P, HCH, 16, DH]),
                        op=MUL,
                    )
                    # q -> channel-major via SBUF->SBUF DMA transpose
                    nc.sync.dma_start_transpose(
                        out=tview(q_cm[:, SLC * s + SLC // 2 * hf:
                                       SLC * s + SLC // 2 * (hf + 1)]),
                        in_=qt[:, es],
                    )
                if DEBUG and blk == DBG_BLK and s == 0:
                    nc.sync.dma_start(dbg_he[:], he[:])
                    nc.sync.dma_start(dbg_et[:], et[:])
                return exps

            def m1_build(blk, G_ps, ks_ps):
                ksC = smal.tile([P, 1], F32, tag="ksC")
                nc.vector.reciprocal(ksC[:], ks_ps[:])
                G_sb = smal.tile([P, P], BF16, tag="Gsb")
                nc.vector.tensor_tensor(G_sb[:], G_ps[:], headmask[:],
                                        op=MUL)
                GT2_ps = tps.tile([P, C], BF16, tag="tiny")
                for b in range(B_LOC):
                    sl = slice(C * b, C * (b + 1))
                    nc.tensor.transpose(
                        GT2_ps[sl, 0:C], G_sb[sl, sl], ident[sl, sl],
                        tile_position=(C * b, C * b),
                    )
                GT2_sb = smal.tile([P, C], BF16, tag="gt2sb")
                nc.vector.tensor_copy(GT2_sb[:], GT2_ps[:])
                M1u_ps = tps.tile([P, C], F32, tag="tiny")
                for b in range(B_LOC):
                    sl = slice(C * b, C * (b + 1))
                    nc.tensor.matmul(
                        M1u_ps[sl, 0:C], GT2_sb[sl, :], W1rep[blk][sl, :],
                        tile_position=(C * b, C * b),
                    )
                M1blk = m1p.tile([P, P], BF16, tag="m1b")
                nc.gpsimd.memset(M1blk[:], 0.0)
                for b in range(B_LOC):
                    sl = slice(C * b, C * (b + 1))
                    nc.vector.tensor_scalar_mul(
                        M1blk[sl, C * b:C * (b + 1)], M1u_ps[sl, 0:C],
                        ksC[sl, :])
                if DEBUG and blk == DBG_BLK:
                    gtmp = smal.tile([P, P], F32, tag="gdump")
                    nc.vector.tensor_copy(gtmp[:], G_ps[:])
                    nc.sync.dma_start(dbg_G[:], gtmp[:])
                    nc.sync.dma_start(dbg_ksC[:], ksC[:])
                    nc.sync.dma_start(dbg_M1[:], M1blk[:])
                return M1blk

            # ================= block pipeline =========================
            # stage_b(blk) for blocks 1,2 is issued inside block blk-1's
            # phase 2; block 0 runs a stage-parallel standalone phase 1.
            G_ps = gps.tile([P, P], F32, tag="G")
            ks_ps = gps.tile([P, 1], F32, tag="ks")
            q_cm = bigq.tile([P, L], BF16, tag="qcm")
            for s in range(NSL):
                stage_b(0, s, he_tiles[s], G_ps, ks_ps, q_cm)

            for blk in range(NB):
                M1blk = m1_build(blk, G_ps, ks_ps)
                last = blk == NB - 1
                if not last:
                    G_next = gps.tile([P, P], F32, tag="G")
                    ks_next = gps.tile([P, 1], F32, tag="ks")
                    q_next = bigq.tile([P, L], BF16, tag="qcm")
                    he_next = []
                    for _s in range(NSL):
                        hx = hex_.tile([P, SLC], BF16, tag="hex")
                        he_next.append(hx)
                # phase 2 (channel-major); next block's phase 1 interleaved
                pending_exps = []
                for t in range(NZ):
                    z1 = zps.tile([P, ZW], F32, tag="z")
                    for hw in range(2):
                        cs = ZW * t + 512 * hw
                        nc.tensor.matmul(
                            z1[:, 512 * hw:512 * (hw + 1)], M1blk[:],
                            q_cm[:, cs:cs + 512],
                        )
                    a1 = a1p.tile([P, ZW], BF16, tag="a1")
                    a1g = nc.scalar.activation(a1[:], z1[:], AF.Gelu,
                                               bias=b1rep[blk][:])
                    for e in pending_exps:
                        add_dep_helper(a1g.ins, e.ins, sync=True,
                                       reason="cluster exp before gelu")
                    pending_exps = []
                    z2 = zps.tile([P, ZW], F32, tag="z")
                    for hw in range(2):
                        nc.tensor.matmul(
                            z2[:, 512 * hw:512 * (hw + 1)], W2blk[blk][:],
                            a1[:, 512 * hw:512 * (hw + 1)],
                        )
                    if t % 2 == 0:
                        hn = hcm.tile([P, 2048], BF16, tag="hcm")
                    ho = hn[:, ZW * (t % 2):ZW * (t % 2 + 1)]
                    if last:
                        g2 = nc.scalar.activation(
                            ho, z2[:], AF.Gelu, bias=b2rep[blk][:],
                            accum_out=pooled_parts[:, t:t + 1],
                        )
                    else:
                        g2 = nc.scalar.activation(
                            ho, z2[:], AF.Gelu, bias=b2rep[blk][:],
                        )
                    if DEBUG and blk == 0 and t == 0:
                        nc.sync.dma_start(dbg_q[:], q_cm[:, 0:SLC])
                    if DEBUG and blk == 0 and t == 1:
                        nc.sync.dma_start(dbg_h1[:], hn[:])
                    if t % 2 == 1 and not last:
                        # next block's token-major h via DMA transpose
                        s = t // 4
                        hh = (t // 2) % 2
                        nc.sync.dma_start_transpose(
                            out=tview(he_next[s][:, 2048 * hh:
                                                 2048 * (hh + 1)]),
                            in_=hn[:],
                        )
                    # grouped next-block phase-1 issue
                    if not last:
                        if t == 7:
                            pending_exps += stage_b(
                                blk + 1, 0, he_next[0], G_next, ks_next,
                                q_next, anchor=g2)
                            pending_exps += stage_b(
                                blk + 1, 1, he_next[1], G_next, ks_next,
                                q_next, anchor=g2)
                        elif t == 11:
                            pending_exps += stage_b(
                                blk + 1, 2, he_next[2], G_next, ks_next,
                                q_next, anchor=g2)
                        elif t == 15:
                            stage_b(blk + 1, 3, he_next[3], G_next, ks_next,
                                    q_next, anchor=g2)
                if not last:
                    G_ps, ks_ps, q_cm = G_next, ks_next, q_next

            # ===================== head =============================
            psum_ = smal.tile([P, 1], F32, tag="poolsum")
            nc.vector.reduce_sum(psum_[:], pooled_parts[:],
                                 axis=mybir.AxisListType.X)
            y_ps = tps.tile([P, C], F32, tag="tiny")
            for b in range(B_LOC):
                sl = slice(C * b, C * (b + 1))
                nc.tensor.matmul(
                    y_ps[sl, 0:1], Whrep[sl, :], psum_[sl, :],
                    tile_position=(C * b, C * b),
                )
            ybn = smal.tile([P, 1], F32, tag="ybn")
            nc.vector.tensor_scalar(
                ybn[:], y_ps[:, 0:1], svecL[:], tvec[:], op0=MUL, op1=ADD,
            )
            yg = smal.tile([P, 1], F32, tag="yg")
            nc.scalar.activation(yg[:], ybn[:], AF.Gelu)
            o_ps = tps.tile([P, C], F32, tag="tiny")
            for b in range(B_LOC):
                nc.tensor.matmul(
                    o_ps[C * b:C * b + DOUT, 0:1],
                    Wfrep[C * b:C * (b + 1), :],
                    yg[C * b:C * (b + 1), :],
                    tile_position=(C * b, C * b),
                )
            ob = smal.tile([P, 1], F32, tag="ob")
            for b in range(B_LOC):
                sl = slice(C * b, C * b + DOUT)
                nc.vector.tensor_tensor(ob[sl, :], o_ps[sl, 0:1], bf_s[sl, :],
                                        op=ADD)
            for b in range(B_LOC):
                nc.sync.dma_start(
                    out_d[b, :], ob[C * b:C * b + DOUT, 0],
                )

    _fix_sync_waits(nc)
    return nc


def _derive_params(arrs):
    """Host-side precompute: replicated weight strips + BN eval folding."""
    tile4 = lambda a: np.tile(a, (B_LOC,) + (1,) * (a.ndim - 1))
    W1 = arrs["fcW1"]            # [NB, C, C]
    W2 = arrs["fcW2"]
    W2b = np.zeros((NB, P, P), np.float32)
    for i in range(NB):
        for b in range(B_LOC):
            W2b[i, C * b:C * (b + 1), C * b:C * (b + 1)] = W2[i]
    svec = arrs["bn_gamma"] / np.sqrt(arrs["bn_var"] + BN_EPS)
    tv = (arrs["bh"] - arrs["bn_mean"]) * svec + arrs["bn_beta"]
    bfr = np.zeros(P, np.float32)
    for b in range(B_LOC):
        bfr[C * b:C * b + DOUT] = arrs["bf"]
    return {
        "W1r": np.ascontiguousarray(
            np.stack([tile4(W1[i]) for i in range(NB)])),
        "W2b": W2b,
        "b1r": np.ascontiguousarray(
            np.stack([tile4(arrs["fcb1"][i]) for i in range(NB)])),
        "b2r": np.ascontiguousarray(
            np.stack([tile4(arrs["fcb2"][i]) for i in range(NB)])),
        "Whr": tile4(arrs["Wh"]),
        "Wfr": tile4(arrs["Wf"]),
        "svecLr": tile4(svec / L),
        "tvecr": tile4(tv),
        "bfr": bfr,
    }


_NC_CACHE = [None]


def kernel(**inputs) -> np.ndarray:
    arrs = {k: np.asarray(v, dtype=np.float32) for k, v in inputs.items()}
    x = arrs["x"]
    B = x.shape[0]
    n_cores = 8
    bl = B // n_cores

    if _NC_CACHE[0] is None:
        _NC_CACHE[0] = build_program()
    nc = _NC_CACHE[0]

    params = _derive_params(arrs)
    in_maps = [
        {"x": np.ascontiguousarray(x[bl * i: bl * (i + 1)]), **params}
        for i in range(n_cores)
    ]
    res = run_bass_kernel_spmd(nc, in_maps, list(range(n_cores))).results
    return np.concatenate([res[i]["out"] for i in range(n_cores)], axis=0)
